# revision 6
# baseline (speedup 1.0000x reference)
"""BatchedDiffPool (2x GAT + softmax assign + pooling) on 8 Trainium2 cores.

Strategy (row-parallel over the 8192 nodes, 1024 rows per core):
  Launch 1 (per core k, SPMD):
    - h1 = x@W1, h2 = x@W2 on device (bf16 matmul, fp32 accum)
    - masked-softmax attention without materializing logits:
        n[j,i] = adj[i,j] * exp(lrelu(f1_i + f2_j) - M)
      computed via exp(lrelu(t)) = max(exp(t), exp(0.2 t)) (exp monotone,
      lrelu(t) = max(t, 0.2t)); f1/f2 are rank-1 factors computed host-side
      in fp64 (tiny: x @ (W a)); M is a global upper bound so exps <= 1.
      Attention is built directly in TRANSPOSED layout [j, i] so the
      attention matmul contracts j on partitions with no on-device
      transposes: the host ships adjT (binary -> exact in bf16).
    - z = elu(att1@h1 + h1_loc) ; assign = elu(att2@h2 + h2_loc)
    - s = softmax(assign) (exact rowmax subtraction)
    - xnext_part = s_loc^T z_loc (partial, host-summed)
  Host: gather s (fp32) -> per-core permuted bf16 copy.
  Launch 2 (per core): y = A_k @ s_full ; anext_part = s_loc^T y.
  Host: sum partials, concat s rows.

All inputs to each core use a core-local row permutation (own rows first)
so the SPMD program is identical across cores.
"""
import numpy as np
import ml_dtypes

import concourse.bass as bass
import concourse.mybir as mybir
import concourse.tile as tile
from concourse import bacc
from concourse.bass import ts
from concourse.bass_utils import run_bass_kernel_spmd

BF = ml_dtypes.bfloat16
F32 = np.float32

P = 128
N = 8192          # nodes
NLOC = 1024       # nodes per core
FEAT = 256        # input features
H1 = 128          # nhid
H2 = 1024         # nnext
JC = N // P       # 64 j-chunks
ICH = NLOC // P   # 8 i-chunks
NCORES = 8

dt = mybir.dt
EXP = mybir.ActivationFunctionType.Exp
ADD = mybir.AluOpType.add
MULT = mybir.AluOpType.mult
MAX = mybir.AluOpType.max
MIN = mybir.AluOpType.min

_CACHE = {}


def _mk_nc():
    return bacc.Bacc("TRN2", target_bir_lowering=False, debug=False,
                     num_devices=NCORES)


def _build_launch1():
    nc = _mk_nc()
    adjT = nc.declare_dram_parameter("adjT", [N, NLOC], dt.bfloat16, isOutput=False)
    xT = nc.declare_dram_parameter("xT", [FEAT, N], dt.bfloat16, isOutput=False)
    w1 = nc.declare_dram_parameter("w1", [FEAT, H1], dt.bfloat16, isOutput=False)
    w2 = nc.declare_dram_parameter("w2", [FEAT, H2], dt.bfloat16, isOutput=False)
    f1e = nc.declare_dram_parameter("f1e", [P, NLOC], dt.float32, isOutput=False)
    f1a = nc.declare_dram_parameter("f1a", [P, NLOC], dt.float32, isOutput=False)
    bue = nc.declare_dram_parameter("bue", [P, JC], dt.float32, isOutput=False)
    bve = nc.declare_dram_parameter("bve", [P, JC], dt.float32, isOutput=False)
    bua = nc.declare_dram_parameter("bua", [P, JC], dt.float32, isOutput=False)
    bva = nc.declare_dram_parameter("bva", [P, JC], dt.float32, isOutput=False)
    s_out = nc.declare_dram_parameter("s_out", [NLOC, H2], dt.float32, isOutput=True)
    xn_out = nc.declare_dram_parameter("xn_out", [H2, H1], dt.float32, isOutput=True)

    h1ext_d = nc.dram_tensor("h1ext_d", [JC, P, H1 + 2], dt.bfloat16)
    h2loc_d = nc.dram_tensor("h2loc_d", [ICH, P, H2], dt.float32)

    with tile.TileContext(nc) as tc:
        with tc.tile_pool(name="persist", bufs=1) as pp:
            h2res = pp.tile([P, JC * H2], dt.bfloat16, tag="h2res")
            f1a_t = pp.tile([P, NLOC], dt.float32, tag="f1a")
            bua_t = pp.tile([P, JC], dt.float32, tag="bua")
            bva_t = pp.tile([P, JC], dt.float32, tag="bva")
            ones_t = pp.tile([P, 2], dt.float32, tag="ones")
            zres = pp.tile([P, ICH * H1], dt.float32, tag="zres")
            R2 = pp.tile([P, 512], dt.float32, tag="R2")

            nc.sync.dma_start(out=f1a_t[:], in_=f1a[:])
            nc.sync.dma_start(out=bua_t[:], in_=bua[:])
            nc.sync.dma_start(out=bva_t[:], in_=bva[:])
            nc.vector.memset(ones_t[:], 1.0)

            mid_ctx = tc.tile_pool(name="mid", bufs=1)
            mid = mid_ctx.__enter__()
            f1e_t = mid.tile([P, NLOC], dt.float32, tag="f1e")
            bue_t = mid.tile([P, JC], dt.float32, tag="bue")
            bve_t = mid.tile([P, JC], dt.float32, tag="bve")
            onesb_t = mid.tile([P, 2], dt.bfloat16, tag="onesb")
            h1loc = mid.tile([P, NLOC], dt.float32, tag="h1loc")
            w1_t = mid.tile([P, 2 * H1], dt.bfloat16, tag="w1t")
            w2_t = mid.tile([P, 2 * H2], dt.bfloat16, tag="w2t")

            nc.sync.dma_start(out=f1e_t[:], in_=f1e[:])
            nc.sync.dma_start(out=bue_t[:], in_=bue[:])
            nc.sync.dma_start(out=bve_t[:], in_=bve[:])
            nc.vector.memset(onesb_t[:], 1.0)
            for kc in range(2):
                nc.sync.dma_start(out=w1_t[:, ts(kc, H1)], in_=w1[ts(kc, P), :])
                nc.sync.dma_start(out=w2_t[:, ts(kc, H2)], in_=w2[ts(kc, P), :])

            # ---------------- Phase 0: h1 / h2 ----------------
            with tc.tile_pool(name="p0", bufs=4) as p0, \
                 tc.tile_pool(name="p0ps", bufs=2, space="PSUM") as p0ps:
                for nchk in range(JC):
                    ph1 = p0ps.tile([P, H1], dt.float32, tag="ph1")
                    ph2 = p0ps.tile([P, H2], dt.float32, tag="ph2")
                    for kc in range(2):
                        xt = p0.tile([P, P], dt.bfloat16, tag="xt")
                        nc.sync.dma_start(out=xt[:], in_=xT[ts(kc, P), ts(nchk, P)])
                        nc.tensor.matmul(ph1[:], xt[:], w1_t[:, ts(kc, H1)],
                                         start=(kc == 0), stop=(kc == 1))
                        nc.tensor.matmul(ph2[:, 0:512], xt[:],
                                         w2_t[:, kc * H2:kc * H2 + 512],
                                         start=(kc == 0), stop=(kc == 1))
                        nc.tensor.matmul(ph2[:, 512:1024], xt[:],
                                         w2_t[:, kc * H2 + 512:(kc + 1) * H2],
                                         start=(kc == 0), stop=(kc == 1))
                    nc.scalar.copy(out=h2res[:, ts(nchk, H2)], in_=ph2[:])
                    h1x = p0.tile([P, H1 + 2], dt.bfloat16, tag="h1x")
                    nc.vector.tensor_copy(out=h1x[:, 0:H1], in_=ph1[:])
                    nc.vector.tensor_copy(out=h1x[:, H1:H1 + 2], in_=onesb_t[:])
                    nc.sync.dma_start(out=h1ext_d[nchk], in_=h1x[:])
                    if nchk < ICH:
                        nc.vector.tensor_copy(out=h1loc[:, ts(nchk, H1)], in_=ph1[:])
                        h2st = p0.tile([P, H2], dt.float32, tag="h2st")
                        nc.vector.tensor_copy(out=h2st[:], in_=ph2[:])
                        nc.sync.dma_start(out=h2loc_d[nchk], in_=h2st[:])

            # ---------------- GAT1 (embed) ----------------
            with tc.tile_pool(name="g1", bufs=3) as g1, \
                 tc.tile_pool(name="g1ps", bufs=1, space="PSUM") as g1ps:
                pu1 = [g1ps.tile([P, H1 + 1], dt.float32, tag=f"pu1_{ic}", name=f"pu1_{ic}")
                       for ic in range(ICH)]
                for jc in range(JC):
                    adjt = g1.tile([P, NLOC], dt.bfloat16, tag="adjt")
                    nc.sync.dma_start(out=adjt[:], in_=adjT[ts(jc, P), :])
                    h1e = g1.tile([P, H1 + 2], dt.bfloat16, tag="h1e")
                    nc.sync.dma_start(out=h1e[:], in_=h1ext_d[jc])
                    u = g1.tile([P, NLOC], dt.bfloat16, tag="u")
                    nc.scalar.activation(u[:], f1e_t[:], EXP,
                                         bias=bue_t[:, jc:jc + 1], scale=1.0)
                    v = g1.tile([P, NLOC], dt.bfloat16, tag="v")
                    nc.scalar.activation(v[:], f1e_t[:], EXP,
                                         bias=bve_t[:, jc:jc + 1], scale=0.2)
                    w = g1.tile([P, NLOC], dt.bfloat16, tag="w")
                    nc.vector.tensor_tensor(out=w[:], in0=u[:], in1=v[:], op=MAX)
                    n1 = g1.tile([P, NLOC], dt.bfloat16, tag="n1")
                    nc.vector.tensor_tensor(out=n1[:], in0=w[:], in1=adjt[:], op=MULT)
                    for ic in range(ICH):
                        nc.tensor.matmul(pu1[ic][:], n1[:, ts(ic, P)],
                                         h1e[:, 0:H1 + 1],
                                         start=(jc == 0), stop=(jc == JC - 1))
                # evac -> z
                with tc.tile_pool(name="g1e", bufs=2) as g1e:
                    for ic in range(ICH):
                        r1i = g1e.tile([P, 1], dt.float32, tag="r1i")
                        nc.vector.reciprocal(r1i[:], pu1[ic][:, H1:H1 + 1])
                        zp = g1e.tile([P, H1], dt.float32, tag="zp")
                        nc.vector.scalar_tensor_tensor(
                            out=zp[:], in0=pu1[ic][:, 0:H1], scalar=r1i[:],
                            in1=h1loc[:, ts(ic, H1)], op0=MULT, op1=ADD)
                        zm = g1e.tile([P, H1], dt.bfloat16, tag="zm")
                        nc.vector.tensor_scalar(zm[:], zp[:], 0.0, None, MIN)
                        ze = g1e.tile([P, H1], dt.float32, tag="ze")
                        nc.scalar.activation(ze[:], zm[:], EXP)
                        t1 = g1e.tile([P, H1], dt.float32, tag="t1")
                        nc.vector.scalar_tensor_tensor(
                            out=t1[:], in0=zp[:], scalar=0.0, in1=ze[:],
                            op0=MAX, op1=ADD)
                        nc.vector.tensor_scalar(zres[:, ts(ic, H1)], t1[:],
                                                -1.0, None, ADD)

            mid_ctx.__exit__(None, None, None)

            # ---------------- GAT2 (assign) halves ----------------
            for h in range(2):
                hoff = h * 512
                nc.vector.memset(R2[:], 0.0)
                with tc.tile_pool(name=f"g2_{h}", bufs=3) as g2:
                    g2ps_ctx = tc.tile_pool(name=f"g2ps_{h}", bufs=1, space="PSUM")
                    g2ps = g2ps_ctx.__enter__()
                    pu2 = [g2ps.tile([P, H2], dt.float32, tag=f"pu2_{i4}", name=f"pu2_{h}_{i4}")
                           for i4 in range(4)]
                    for jc in range(JC):
                        adjt = g2.tile([P, 512], dt.bfloat16, tag="adjt2")
                        nc.sync.dma_start(out=adjt[:],
                                          in_=adjT[ts(jc, P), hoff:hoff + 512])
                        u = g2.tile([P, 512], dt.bfloat16, tag="u2")
                        nc.scalar.activation(u[:], f1a_t[:, hoff:hoff + 512], EXP,
                                             bias=bua_t[:, jc:jc + 1], scale=1.0)
                        v = g2.tile([P, 512], dt.bfloat16, tag="v2")
                        nc.scalar.activation(v[:], f1a_t[:, hoff:hoff + 512], EXP,
                                             bias=bva_t[:, jc:jc + 1], scale=0.2)
                        w = g2.tile([P, 512], dt.bfloat16, tag="w2")
                        nc.vector.tensor_tensor(out=w[:], in0=u[:], in1=v[:], op=MAX)
                        n2 = g2.tile([P, 512], dt.bfloat16, tag="n2")
                        nc.vector.tensor_tensor(out=n2[:], in0=w[:], in1=adjt[:],
                                                op=MULT)
                        nc.gpsimd.tensor_tensor(out=R2[:], in0=R2[:], in1=n2[:],
                                                op=ADD)
                        h2j = h2res[:, ts(jc, H2)]
                        for i4 in range(4):
                            nc.tensor.matmul(pu2[i4][:, 0:512], n2[:, ts(i4, P)],
                                             h2j[:, 0:512],
                                             start=(jc == 0), stop=(jc == JC - 1))
                            nc.tensor.matmul(pu2[i4][:, 512:1024], n2[:, ts(i4, P)],
                                             h2j[:, 512:1024],
                                             start=(jc == 0), stop=(jc == JC - 1))
                    # evacuate psum -> araw (so banks free for r2 reduction)
                    with tc.tile_pool(name=f"g2a_{h}", bufs=1) as g2a:
                        araw = []
                        for i4 in range(4):
                            a_t = g2a.tile([P, H2], dt.float32, tag=f"araw{i4}")
                            nc.scalar.copy(out=a_t[:], in_=pu2[i4][:])
                            araw.append(a_t)
                        g2ps_ctx.__exit__(None, None, None)
                        # r2 = colsum of R2 (partition reduce via PE)
                        with tc.tile_pool(name=f"r2ps_{h}", bufs=1,
                                          space="PSUM") as r2ps, \
                             tc.tile_pool(name=f"g2f_{h}", bufs=1) as g2f:
                            for i4 in range(4):
                                ic = h * 4 + i4
                                r2p = r2ps.tile([P, 1], dt.float32, tag=f"r2p{i4}")
                                nc.tensor.matmul(r2p[:], R2[:, ts(i4, P)],
                                                 ones_t[:, 0:1],
                                                 start=True, stop=True)
                                r2i = g2f.tile([P, 1], dt.float32, tag="r2i")
                                nc.vector.reciprocal(r2i[:], r2p[:])
                                h2l = g2f.tile([P, H2], dt.float32, tag="h2l", bufs=2)
                                nc.sync.dma_start(out=h2l[:], in_=h2loc_d[ic])
                                a_f = g2f.tile([P, H2], dt.float32, tag="a_f")
                                nc.vector.scalar_tensor_tensor(
                                    out=a_f[:], in0=araw[i4][:], scalar=r2i[:],
                                    in1=h2l[:], op0=MULT, op1=ADD)
                                # assign' = exp(min(a,0)) + relu(a)  (elu + 1)
                                am = g2f.tile([P, H2], dt.bfloat16, tag="am")
                                nc.vector.tensor_scalar(am[:], a_f[:], 0.0, None, MIN)
                                ex = g2f.tile([P, H2], dt.float32, tag="ex")
                                nc.scalar.activation(ex[:], am[:], EXP)
                                asg = g2f.tile([P, H2], dt.float32, tag="asg")
                                nc.vector.scalar_tensor_tensor(
                                    out=asg[:], in0=a_f[:], scalar=0.0, in1=ex[:],
                                    op0=MAX, op1=ADD)
                                # softmax with exact rowmax
                                mx = g2f.tile([P, 1], dt.float32, tag="mx")
                                nc.vector.tensor_reduce(mx[:], asg[:],
                                                        mybir.AxisListType.X, MAX)
                                nmx = g2f.tile([P, 1], dt.float32, tag="nmx")
                                nc.vector.tensor_scalar(nmx[:], mx[:], -1.0, None,
                                                        MULT)
                                pexp = g2f.tile([P, H2], dt.float32, tag="pexp")
                                rs = g2f.tile([P, 1], dt.float32, tag="rs")
                                nc.scalar.activation(pexp[:], asg[:], EXP,
                                                     bias=nmx[:], scale=1.0,
                                                     accum_out=rs[:])
                                rsi = g2f.tile([P, 1], dt.float32, tag="rsi")
                                nc.vector.reciprocal(rsi[:], rs[:])
                                s_t = g2f.tile([P, H2], dt.float32, tag="s_t", bufs=2)
                                nc.vector.tensor_scalar(s_t[:], pexp[:], rsi[:],
                                                        None, MULT)
                                nc.sync.dma_start(out=s_out[ts(ic, P), :],
                                                  in_=s_t[:])

            # ---------------- xnext partial = s_loc^T z ----------------
            with tc.tile_pool(name="xn", bufs=3) as xn, \
                 tc.tile_pool(name="xnps", bufs=1, space="PSUM") as xnps:
                px = [xnps.tile([P, H1], dt.float32, tag=f"px{cc}", name=f"px{cc}")
                      for cc in range(ICH)]
                for ic in range(ICH):
                    st = xn.tile([P, H2], dt.float32, tag="sread")
                    nc.sync.dma_start(out=st[:], in_=s_out[ts(ic, P), :])
                    for cc in range(ICH):
                        nc.tensor.matmul(px[cc][:], st[:, ts(cc, P)],
                                         zres[:, ts(ic, H1)],
                                         start=(ic == 0), stop=(ic == ICH - 1))
                for cc in range(ICH):
                    xo = xn.tile([P, H1], dt.float32, tag="xo")
                    nc.vector.tensor_copy(out=xo[:], in_=px[cc][:])
                    nc.sync.dma_start(out=xn_out[ts(cc, P), :], in_=xo[:])

    nc.compile()
    return nc


def _build_launch2():
    nc = _mk_nc()
    adjT = nc.declare_dram_parameter("adjT", [N, NLOC], dt.bfloat16, isOutput=False)
    sfull = nc.declare_dram_parameter("sfull", [N, H2], dt.bfloat16, isOutput=False)
    an_out = nc.declare_dram_parameter("an_out", [H2, H2], dt.float32, isOutput=True)

    with tile.TileContext(nc) as tc:
        with tc.tile_pool(name="persist", bufs=1) as pp:
            y_sb = pp.tile([P, ICH * H2], dt.bfloat16, tag="y_sb")
            s_loc = pp.tile([P, ICH * H2], dt.bfloat16, tag="s_loc")
            for ic in range(ICH):
                nc.sync.dma_start(out=s_loc[:, ts(ic, H2)], in_=sfull[ts(ic, P), :])

            for h in range(2):
                ihoff = h * 512
                with tc.tile_pool(name=f"ph_{h}", bufs=3) as ph, \
                     tc.tile_pool(name=f"phps_{h}", bufs=1, space="PSUM") as phps:
                    py = [phps.tile([P, H2], dt.float32, tag=f"py{i4}", name=f"py{h}_{i4}")
                          for i4 in range(4)]
                    for jc in range(JC):
                        adjt = ph.tile([P, 512], dt.bfloat16, tag="adjt")
                        nc.sync.dma_start(out=adjt[:],
                                          in_=adjT[ts(jc, P), ihoff:ihoff + 512])
                        sj = ph.tile([P, H2], dt.bfloat16, tag="sj")
                        nc.sync.dma_start(out=sj[:], in_=sfull[ts(jc, P), :])
                        for i4 in range(4):
                            nc.tensor.matmul(py[i4][:, 0:512], adjt[:, ts(i4, P)],
                                             sj[:, 0:512],
                                             start=(jc == 0), stop=(jc == JC - 1))
                            nc.tensor.matmul(py[i4][:, 512:1024], adjt[:, ts(i4, P)],
                                             sj[:, 512:1024],
                                             start=(jc == 0), stop=(jc == JC - 1))
                    for i4 in range(4):
                        ic = h * 4 + i4
                        nc.scalar.copy(out=y_sb[:, ts(ic, H2)], in_=py[i4][:])

            # anext_part = s_loc^T y
            with tc.tile_pool(name="an", bufs=3) as an, \
                 tc.tile_pool(name="anps", bufs=2, space="PSUM") as anps:
                for cc in range(ICH):
                    pa = anps.tile([P, H2], dt.float32, tag="pa")
                    for ic in range(ICH):
                        nc.tensor.matmul(pa[:, 0:512],
                                         s_loc[:, ic * H2 + cc * P:
                                               ic * H2 + (cc + 1) * P],
                                         y_sb[:, ic * H2:ic * H2 + 512],
                                         start=(ic == 0), stop=(ic == ICH - 1))
                        nc.tensor.matmul(pa[:, 512:1024],
                                         s_loc[:, ic * H2 + cc * P:
                                               ic * H2 + (cc + 1) * P],
                                         y_sb[:, ic * H2 + 512:(ic + 1) * H2],
                                         start=(ic == 0), stop=(ic == ICH - 1))
                    ao = an.tile([P, H2], dt.float32, tag="ao")
                    nc.vector.tensor_copy(out=ao[:], in_=pa[:])
                    nc.sync.dma_start(out=an_out[ts(cc, P), :], in_=ao[:])

    nc.compile()
    return nc


def _get(name, builder):
    if name not in _CACHE:
        _CACHE[name] = builder()
    return _CACHE[name]


def _host_prep(x, adj, W_embed, a_embed, W_assign, a_assign):
    x64 = np.asarray(x, np.float64)
    W1 = np.asarray(W_embed, np.float64)
    a1 = np.asarray(a_embed, np.float64)
    W2 = np.asarray(W_assign, np.float64)
    a2 = np.asarray(a_assign, np.float64)

    h1_64 = x64 @ W1
    h2_64 = x64 @ W2
    f1e = (h1_64 @ a1[:H1, 0]).astype(F32)
    f2e = (h1_64 @ a1[H1:, 0]).astype(F32)
    f1a = (h2_64 @ a2[:H2, 0]).astype(F32)
    f2a = (h2_64 @ a2[H2:, 0]).astype(F32)
    Me = float(f1e.max() + f2e.max())
    Ma = float(f1a.max() + f2a.max())

    adjF = np.ascontiguousarray(np.asarray(adj, F32).T).astype(BF)  # [j, i] 8192^2
    xTb = np.ascontiguousarray(x64.T).astype(BF)                    # [feat, n]
    w1b = np.asarray(W_embed, F32).astype(BF)
    w2b = np.asarray(W_assign, F32).astype(BF)

    def roll_rows(a, k):
        kb = k * NLOC
        return np.concatenate([a[kb:kb + NLOC], a[:kb], a[kb + NLOC:]], axis=0)

    in_maps = []
    for k in range(NCORES):
        kb = k * NLOC
        adjT_k = np.ascontiguousarray(roll_rows(adjF, k)[:, kb:kb + NLOC])
        xT_k = np.ascontiguousarray(
            np.concatenate([xTb[:, kb:kb + NLOC], xTb[:, :kb], xTb[:, kb + NLOC:]],
                           axis=1))
        f2e_p = roll_rows(f2e, k)
        f2a_p = roll_rows(f2a, k)
        m = {
            "adjT": adjT_k,
            "xT": xT_k,
            "w1": w1b,
            "w2": w2b,
            "f1e": np.ascontiguousarray(
                np.broadcast_to(f1e[kb:kb + NLOC], (P, NLOC))).astype(F32),
            "f1a": np.ascontiguousarray(
                np.broadcast_to(f1a[kb:kb + NLOC], (P, NLOC))).astype(F32),
            "bue": np.ascontiguousarray((f2e_p - Me).reshape(JC, P).T),
            "bve": np.ascontiguousarray((0.2 * f2e_p - Me).reshape(JC, P).T),
            "bua": np.ascontiguousarray((f2a_p - Ma).reshape(JC, P).T),
            "bva": np.ascontiguousarray((0.2 * f2a_p - Ma).reshape(JC, P).T),
        }
        in_maps.append(m)
    return in_maps


def kernel(x, adj, W_embed, a_embed, W_assign, a_assign, _profile=None):
    in_maps1 = _host_prep(x, adj, W_embed, a_embed, W_assign, a_assign)

    nc1 = _get("l1", _build_launch1)
    res1 = run_bass_kernel_spmd(nc1, in_maps1, list(range(NCORES)),
                                **(_profile or {}).get("l1", {}))
    if _profile is not None:
        _profile["res1"] = res1

    s_full = np.concatenate([res1.results[k]["s_out"] for k in range(NCORES)],
                            axis=0)                      # [N, H2] fp32
    xnext = np.sum([res1.results[k]["xn_out"] for k in range(NCORES)], axis=0)

    s_bf = s_full.astype(BF)
    in_maps2 = []
    for k in range(NCORES):
        kb = k * NLOC
        sf_k = np.concatenate([s_bf[kb:kb + NLOC], s_bf[:kb], s_bf[kb + NLOC:]],
                              axis=0)
        in_maps2.append({"adjT": in_maps1[k]["adjT"], "sfull": sf_k})

    nc2 = _get("l2", _build_launch2)
    res2 = run_bass_kernel_spmd(nc2, in_maps2, list(range(NCORES)),
                                **(_profile or {}).get("l2", {}))
    if _profile is not None:
        _profile["res2"] = res2

    anext = np.sum([res2.results[k]["an_out"] for k in range(NCORES)], axis=0)

    return (xnext.astype(F32), anext.astype(F32), s_full.astype(F32))
